# revision 1
# baseline (speedup 1.0000x reference)
"""Data-parallel CentroidEstimationModel kernel for 8 NeuronCores.

Sharding: pure data parallel over the cluster/batch dim B=4096 -> 8 shards
of 512. All params are tiny and replicated to every core. Each shard is
dispatched asynchronously to its own NeuronCore; results are gathered on
host into the full [4096, 512] output.
"""
import numpy as np
import jax
import jax.numpy as jnp

B, N, D, NH, P = 4096, 30, 512, 4, 30
M = 8  # NeuronCores

_EPS = 1e-8


def _ln(x, g, b, eps=1e-5):
    mu = jnp.mean(x, axis=-1, keepdims=True)
    var = jnp.mean((x - mu) ** 2, axis=-1, keepdims=True)
    return g * (x - mu) / jnp.sqrt(var + eps) + b


def _forward(x, attention_mask, order, num_docs, docs_weights,
             pos_emb, fc1_w1, fc1_b1, fc1_w2, fc1_b2, fc2_w, fc2_b,
             ln1_g, ln1_b, ln2_g, ln2_b, ln3_g, ln3_b, ln4_g, ln4_b):
    xn = x / jnp.maximum(jnp.linalg.norm(x, axis=2, keepdims=True), _EPS)
    x1 = _ln(xn, ln1_g, ln1_b)
    xp = x1 + pos_emb[order]
    mp = jnp.sum(xp * docs_weights[:, :, None], axis=1, keepdims=True) / num_docs[:, None, None]
    num = jnp.sum(mp * xp, axis=2)
    den = jnp.maximum(jnp.linalg.norm(mp, axis=2) * jnp.linalg.norm(xp, axis=2), _EPS)
    cos = (num / den)[:, :, None]
    fc1_in = jnp.concatenate([xp, jnp.broadcast_to(mp, xp.shape), cos], axis=2)
    Z = jnp.tanh(fc1_in @ fc1_w1 + fc1_b1) @ fc1_w2 + fc1_b2
    Z = jnp.where(attention_mask[:, :, None], -jnp.inf, Z)
    A = jax.nn.softmax(Z, axis=1)
    b, n, h = A.shape
    d = x1.shape[2]
    A_h = A.reshape(b, h, n)  # faithful reshape (not a transpose)
    Hh = jnp.einsum('bhn,bnd->bhd', A_h, x1).reshape(b, h * d)
    mpx = jnp.sum(x1 * docs_weights[:, :, None], axis=1) / num_docs[:, None]
    Hh = _ln(Hh + jnp.tile(mpx, (1, h)), ln2_g, ln2_b)
    pred = _ln(Hh @ fc2_w + fc2_b, ln3_g, ln3_b)
    pred = _ln(pred + jnp.mean(Hh.reshape(b, h, d), axis=1), ln4_g, ln4_b)
    return pred


_jitted = jax.jit(_forward)


def kernel(x, attention_mask, order, num_docs, docs_weights, clusters_centroids,
           pos_emb, fc1_w1, fc1_b1, fc1_w2, fc1_b2, fc2_w, fc2_b,
           ln1_g, ln1_b, ln2_g, ln2_b, ln3_g, ln3_b, ln4_g, ln4_b):
    devs = jax.devices()[:M]
    bs = x.shape[0] // M

    # order is an index tensor; keep it integral (int32 is enough for P=30)
    order = np.asarray(order).astype(np.int32)
    attention_mask = np.asarray(attention_mask)

    params = (pos_emb, fc1_w1, fc1_b1, fc1_w2, fc1_b2, fc2_w, fc2_b,
              ln1_g, ln1_b, ln2_g, ln2_b, ln3_g, ln3_b, ln4_g, ln4_b)

    # Replicate params onto each core, shard the batch dim. H2D placement is
    # issued from one thread per core so the 8 shard transfers overlap; the
    # jit dispatches are async, so all 8 cores compute concurrently.
    from concurrent.futures import ThreadPoolExecutor

    def _place(i):
        dev = devs[i]
        s = slice(i * bs, (i + 1) * bs)
        return (
            jax.device_put(np.asarray(x[s]), dev),
            jax.device_put(attention_mask[s], dev),
            jax.device_put(order[s], dev),
            jax.device_put(np.asarray(num_docs[s]), dev),
            jax.device_put(np.asarray(docs_weights[s]), dev),
        ) + tuple(jax.device_put(np.asarray(p), dev) for p in params)

    with ThreadPoolExecutor(max_workers=M) as ex:
        per_dev_args = list(ex.map(_place, range(M)))

    outs = [_jitted(*args) for args in per_dev_args]
    out = np.concatenate([np.asarray(o) for o in outs], axis=0)
    return out.astype(np.float32)

